# revision 1
# baseline (speedup 1.0000x reference)
"""Causal self-attention (B=4, T=2048, C=1024, H=16) on 8 TRN2 NeuronCores.

Sharding: tensor-parallel over heads. Core c owns heads (2c, 2c+1) for all
batches: QKV projections are column-sharded, attention is embarrassingly
parallel over (batch, head), out_proj is contraction-sharded and the host
sums the 8 partial outputs (the unshard step for a contraction shard).

Per-core kernel layout (all GEMMs bf16 operands, fp32 PSUM accumulation):
  - x^T [C, B*T] bf16 resident in DRAM (host pre-transposed/bf16-rounded,
    matching the reference's x.astype(bf16) rounding).
  - Q^T/K^T computed feature-major [128, B*T]: W-block stationary, x^T moving.
    RoPE: weights are host-permuted so each head's even dims land in
    partitions [0:32) and odd dims in [32:64). Then
      t_a = (q + bq) * [cos|sin|cos|sin],  t_b = (q + bq) * [sin|cos|sin|cos]
    (two full-width DVE ops), and the rotation combine
      rot = Ca^T t_a + Cb^T t_b
    is two bf16 PE matmuls with +-1 matrices.
  - V feature-major then PE-transposed into token-major 128-blocks with an
    appended ones column: the PV matmul then emits both O^T rows and the
    softmax denominator row in one accumulation.
  - S^T [k, q] tiles: matmul(lhsT=K^T block, rhs=Q^T tile). Softmax skips
    max-subtraction (|S|/8 <= ~3.1 for this operator, exp <= ~21), exp runs
    on ACT over 2 k-blocks per instruction, the causal staircase is a
    precomputed bf16 0/1 mask multiply on the diagonal groups only, and
    fully-masked k-blocks are never computed.
  - O rows are normalized by the denominator row (reciprocal + gpsimd
    partition_broadcast + DVE multiply) into y^T bf16 (matching the
    reference's bf16 rounding of y before out_proj).
  - out_proj: wo-block stationary, y^T moving -> partial out^T [C, B*T] fp32.

Host: sums the 8 partials, adds bo, transposes back to (B, T, C).
"""

import numpy as np
import ml_dtypes

import concourse.bass as bass
import concourse.mybir as mybir
import concourse.tile as tile
from concourse import bacc
from concourse.bass_utils import run_bass_kernel_spmd
from concourse.masks import make_identity

BF16 = mybir.dt.bfloat16
F32 = mybir.dt.float32
AT = mybir.ActivationFunctionType
OP = mybir.AluOpType

B, T, C, H = 4, 2048, 1024, 16
DH = 64
BT = B * T            # 8192
NCORES = 8
NTT = BT // 512       # 16 token tiles of 512
NKB = T // 128        # 16 k-blocks per batch

_NC = None            # cached compiled Bass module


def _build_nc():
    nc = bacc.Bacc("TRN2", target_bir_lowering=False, debug=False)

    xT = nc.declare_dram_parameter("xT", [C, BT], BF16, isOutput=False)
    wq = nc.declare_dram_parameter("wq", [C, 128], BF16, isOutput=False)
    wk = nc.declare_dram_parameter("wk", [C, 128], BF16, isOutput=False)
    wv = nc.declare_dram_parameter("wv", [C, 128], BF16, isOutput=False)
    wo = nc.declare_dram_parameter("wo", [128, C], BF16, isOutput=False)
    bq = nc.declare_dram_parameter("bq", [128, 1], F32, isOutput=False)
    bk = nc.declare_dram_parameter("bk", [128, 1], F32, isOutput=False)
    bv = nc.declare_dram_parameter("bv", [128, 1], F32, isOutput=False)
    csa = nc.declare_dram_parameter("csa", [128, T], F32, isOutput=False)
    csb = nc.declare_dram_parameter("csb", [128, T], F32, isOutput=False)
    msk = nc.declare_dram_parameter("msk", [128, 4, 512], BF16, isOutput=False)
    ca = nc.declare_dram_parameter("ca", [128, 128], BF16, isOutput=False)
    cb = nc.declare_dram_parameter("cb", [128, 128], BF16, isOutput=False)
    outT = nc.declare_dram_parameter("outT", [C, BT], F32, isOutput=True)

    from contextlib import ExitStack
    with tile.TileContext(nc) as tc, ExitStack() as ctx:
        const = ctx.enter_context(tc.tile_pool(name="const", bufs=1))
        xpool = ctx.enter_context(tc.tile_pool(name="xpool", bufs=10))
        ptp = ctx.enter_context(tc.tile_pool(name="ptp", bufs=4))
        rtmp = ctx.enter_context(tc.tile_pool(name="rtmp", bufs=4))
        small = ctx.enter_context(tc.tile_pool(name="small", bufs=3))
        psA = ctx.enter_context(tc.tile_pool(name="psA", bufs=2, space="PSUM"))
        psT = ctx.enter_context(tc.tile_pool(name="psT", bufs=1, space="PSUM"))
        psS = ctx.enter_context(tc.tile_pool(name="psS", bufs=2, space="PSUM"))
        psO = ctx.enter_context(tc.tile_pool(name="psO", bufs=1, space="PSUM"))

        # ---- constants ----
        wq_sb = const.tile([128, 8, 128], BF16, tag="wq")
        nc.sync.dma_start(out=wq_sb, in_=wq.rearrange("(kb p) m -> p kb m", p=128))
        wk_sb = const.tile([128, 8, 128], BF16, tag="wk")
        nc.sync.dma_start(out=wk_sb, in_=wk.rearrange("(kb p) m -> p kb m", p=128))
        wv_sb = const.tile([128, 8, 128], BF16, tag="wv")
        nc.sync.dma_start(out=wv_sb, in_=wv.rearrange("(kb p) m -> p kb m", p=128))
        wo_sb = const.tile([128, 8, 128], BF16, tag="wo")
        nc.sync.dma_start(out=wo_sb, in_=wo.rearrange("p (ob m) -> p ob m", m=128))
        csa_sb = const.tile([128, T], F32, tag="csa")
        nc.sync.dma_start(out=csa_sb, in_=csa[:, :])
        csb_sb = const.tile([128, T], F32, tag="csb")
        nc.sync.dma_start(out=csb_sb, in_=csb[:, :])
        msk_sb = const.tile([128, 4, 512], BF16, tag="msk")
        nc.sync.dma_start(out=msk_sb, in_=msk[:, :, :])
        ca_sb = const.tile([128, 128], BF16, tag="ca")
        nc.sync.dma_start(out=ca_sb, in_=ca[:, :])
        cb_sb = const.tile([128, 128], BF16, tag="cb")
        nc.sync.dma_start(out=cb_sb, in_=cb[:, :])
        bq_sb = const.tile([128, 1], F32, tag="bq")
        nc.sync.dma_start(out=bq_sb, in_=bq[:, :])
        bk_sb = const.tile([128, 1], F32, tag="bk")
        nc.sync.dma_start(out=bk_sb, in_=bk[:, :])
        bv_sb = const.tile([128, 1], F32, tag="bv")
        nc.sync.dma_start(out=bv_sb, in_=bv[:, :])

        ident = const.tile([128, 64], BF16, tag="id")
        make_identity(nc, ident[0:64, :])
        make_identity(nc, ident[64:128, :])

        QT = const.tile([128, BT], BF16, tag="QT")
        KT = const.tile([128, BT], BF16, tag="KT")
        VT = const.tile([128, BT], BF16, tag="VT")
        yT = const.tile([128, BT], BF16, tag="yT")
        # token-major V blocks, col 64 = ones (denominator trick), col 65 pad
        vtm = const.tile([128, 2, B * NKB, 66], BF16, tag="vtm")
        nc.vector.memset(vtm[:, :, :, 64:65], 1.0)

        # ---- phase B: projections + RoPE + V transpose ----
        for tt in range(NTT):
            ts_ = slice(tt * 512, tt * 512 + 512)
            pos = slice((tt % 4) * 512, (tt % 4) * 512 + 512)
            xts = []
            for kb in range(8):
                xt = xpool.tile([128, 512], BF16, tag="xt", name=f"xt_{tt}_{kb}")
                nc.sync.dma_start(out=xt, in_=xT[kb * 128:(kb + 1) * 128, ts_])
                xts.append(xt)
            for w_sb, b_sb, dstT, rope in (
                (wq_sb, bq_sb, QT, True),
                (wk_sb, bk_sb, KT, True),
                (wv_sb, bv_sb, VT, False),
            ):
                pp = psA.tile([128, 512], F32, tag="proj", name=f"pp_{tt}")
                for kb in range(8):
                    nc.tensor.matmul(
                        pp, w_sb[:, kb, :], xts[kb], start=(kb == 0), stop=(kb == 7)
                    )
                if not rope:
                    nc.scalar.activation(VT[:, ts_], pp, AT.Identity,
                                         bias=bv_sb[:, 0:1])
                else:
                    ta = rtmp.tile([128, 512], BF16, tag="ta", name=f"ta_{tt}")
                    tb = rtmp.tile([128, 512], BF16, tag="tb", name=f"tb_{tt}")
                    nc.vector.scalar_tensor_tensor(
                        out=ta, in0=pp, scalar=b_sb[:, 0:1], in1=csa_sb[:, pos],
                        op0=OP.add, op1=OP.mult,
                    )
                    nc.vector.scalar_tensor_tensor(
                        out=tb, in0=pp, scalar=b_sb[:, 0:1], in1=csb_sb[:, pos],
                        op0=OP.add, op1=OP.mult,
                    )
                    rp = psA.tile([128, 512], F32, tag="proj", name=f"rp_{tt}")
                    nc.tensor.matmul(rp, ca_sb, ta, start=True, stop=False)
                    nc.tensor.matmul(rp, cb_sb, tb, start=False, stop=True)
                    nc.scalar.copy(dstT[:, ts_], rp)
            # V^T -> token-major blocks
            for j in range(2):
                for sub in range(4):
                    gkb = tt * 4 + sub
                    col = slice(tt * 512 + sub * 128, tt * 512 + sub * 128 + 128)
                    tp = psT.tile([128, 64], BF16, tag="tr", name=f"tp_{tt}")
                    nc.tensor.transpose(
                        tp, VT[64 * j:64 * j + 64, col], ident[64 * j:64 * j + 64, :]
                    )
                    nc.vector.tensor_copy(vtm[:, j, gkb, 0:64], tp)

        # ---- phase C: attention per (batch, head, q-tile) ----
        for b in range(B):
            for j in range(2):
                hsl = slice(64 * j, 64 * j + 64)
                for qt in range(4):
                    qsl = slice(b * T + qt * 512, b * T + qt * 512 + 512)
                    nkb = 4 * qt + 4          # k-blocks covering k <= q0+511
                    ngrp = nkb // 2
                    op_ = psO.tile([128, 512], F32, tag="o", name=f"o_{b}_{j}_{qt}")
                    for g in range(ngrp):
                        sp = psS.tile([128, 2, 512], F32, tag="s",
                                      name=f"s_{b}_{j}_{qt}_{g}")
                        for i in range(2):
                            kb = 2 * g + i
                            ksl = slice(b * T + kb * 128, b * T + kb * 128 + 128)
                            nc.tensor.matmul(
                                sp[:, i, :], KT[hsl, ksl], QT[hsl, qsl],
                                start=True, stop=True,
                            )
                        pt = ptp.tile([128, 2, 512], BF16, tag="pt",
                                      name=f"pt_{b}_{j}_{qt}_{g}")
                        nc.scalar.activation(pt, sp, AT.Exp, scale=0.125)
                        if g >= ngrp - 2:
                            p_idx = g - (ngrp - 2)
                            nc.vector.tensor_tensor(
                                out=pt, in0=pt,
                                in1=msk_sb[:, 2 * p_idx:2 * p_idx + 2, :],
                                op=OP.mult,
                            )
                        for i in range(2):
                            kb = 2 * g + i
                            nc.tensor.matmul(
                                op_[0:65, :], vtm[:, j, b * NKB + kb, 0:65],
                                pt[:, i, :],
                                start=(kb == 0), stop=(kb == nkb - 1),
                            )
                    recip = small.tile([1, 512], F32, tag="rc",
                                       name=f"rc_{b}_{j}_{qt}")
                    nc.vector.reciprocal(recip, op_[64:65, :])
                    rbt = small.tile([64, 512], F32, tag="rb",
                                     name=f"rb_{b}_{j}_{qt}")
                    nc.gpsimd.partition_broadcast(rbt, recip)
                    nc.vector.tensor_tensor(
                        out=yT[hsl, qsl], in0=op_[0:64, :], in1=rbt, op=OP.mult
                    )

        # ---- phase D: out_proj partials ----
        for tt in range(NTT):
            ts_ = slice(tt * 512, tt * 512 + 512)
            for ob in range(8):
                pp = psA.tile([128, 512], F32, tag="proj", name=f"op_{tt}_{ob}")
                nc.tensor.matmul(pp, wo_sb[:, ob, :], yT[:, ts_],
                                 start=True, stop=True)
                ot = small.tile([128, 512], F32, tag="ot", name=f"ot_{tt}_{ob}")
                nc.vector.tensor_copy(ot, pp)
                nc.sync.dma_start(out=outT[ob * 128:(ob + 1) * 128, ts_], in_=ot)

    nc.compile()
    return nc


def _get_nc():
    global _NC
    if _NC is None:
        _NC = _build_nc()
    return _NC


def _prep_in_maps(x, Wq, bq, Wk, bk, Wv, bv, Wo, bo):
    bf = ml_dtypes.bfloat16
    # x^T, bf16-rounded (matches reference's x.astype(bf16) exactly)
    xT = np.ascontiguousarray(
        np.asarray(x, np.float32).reshape(BT, C).astype(bf).T
    )

    # RoPE caches; rows [cos|sin|cos|sin] and [sin|cos|sin|cos]
    inv = (1.0 / 10000.0 ** (np.arange(0, DH, 2, dtype=np.float64) / DH))
    pos = np.arange(T, dtype=np.float64)
    fr = np.outer(pos, inv)                      # [T, 32]
    cosT = np.cos(fr).T.astype(np.float32)       # [32, T]
    sinT = np.sin(fr).T.astype(np.float32)
    csa = np.ascontiguousarray(np.concatenate([cosT, sinT, cosT, sinT], 0))
    csb = np.ascontiguousarray(np.concatenate([sinT, cosT, sinT, cosT], 0))

    # causal staircase masks for the 4 diagonal k-blocks of each q-tile
    ki = np.arange(128)[:, None]
    qi = np.arange(512)[None, :]
    msk = np.stack(
        [(qi >= 128 * jj + ki) for jj in range(4)], axis=1
    ).astype(bf)                                  # [128, 4, 512]

    # RoPE combine matrices: rot = Ca^T t_a + Cb^T t_b
    ca = np.zeros((128, 128), np.float32)
    cb = np.zeros((128, 128), np.float32)
    for base in (0, 64):
        for m in range(32):
            ca[base + m, base + m] = 1.0          # E*cos
            ca[base + m + 32, base + m] = -1.0    # -O*sin
            cb[base + m, base + m + 32] = 1.0     # E*sin
            cb[base + m + 32, base + m + 32] = 1.0  # O*cos
    ca = ca.astype(bf)
    cb = cb.astype(bf)

    perm = np.concatenate([np.arange(0, DH, 2), np.arange(1, DH, 2)])
    Wq = np.asarray(Wq, np.float32)
    Wk = np.asarray(Wk, np.float32)
    Wv = np.asarray(Wv, np.float32)
    Wo = np.asarray(Wo, np.float32)
    bq = np.asarray(bq, np.float32)
    bk = np.asarray(bk, np.float32)
    bv = np.asarray(bv, np.float32)

    in_maps = []
    for c in range(NCORES):
        h0, h1 = 2 * c, 2 * c + 1
        cols = np.concatenate([DH * h0 + perm, DH * h1 + perm])
        in_maps.append({
            "xT": xT,
            "wq": np.ascontiguousarray(Wq[:, cols].astype(bf)),
            "wk": np.ascontiguousarray(Wk[:, cols].astype(bf)),
            "wv": np.ascontiguousarray(Wv[:, 128 * c:128 * c + 128].astype(bf)),
            "wo": np.ascontiguousarray(Wo[128 * c:128 * c + 128, :].astype(bf)),
            "bq": np.ascontiguousarray(bq[cols].reshape(128, 1)),
            "bk": np.ascontiguousarray(bk[cols].reshape(128, 1)),
            "bv": np.ascontiguousarray(
                bv[128 * c:128 * c + 128].reshape(128, 1)),
            "csa": csa, "csb": csb, "msk": msk, "ca": ca, "cb": cb,
        })
    return in_maps


def _gather(results, bo):
    acc = results[0]["outT"].astype(np.float32)
    for c in range(1, NCORES):
        acc = acc + results[c]["outT"]
    out = acc.T.reshape(B, T, C) + np.asarray(bo, np.float32)
    return np.ascontiguousarray(out.astype(np.float32))


def kernel(x, Wq, bq, Wk, bk, Wv, bv, Wo, bo):
    nc = _get_nc()
    in_maps = _prep_in_maps(x, Wq, bq, Wk, bk, Wv, bv, Wo, bo)
    res = run_bass_kernel_spmd(nc, in_maps, list(range(NCORES)))
    return _gather(res.results, bo)
